# revision 9
# baseline (speedup 1.0000x reference)
"""Trainium2 Bass kernel for windowed 3D attention (nn_Attention_12927851561046).

512 windows of 343-token, 4-head, 32-dim-per-head attention over d=128.
Pure data parallel: 64 windows per core across 8 NeuronCores.

Layout strategy (per window):
  xt (d=128 partitions, 384 tokens padded) bf16
  qT/kT = w^T@xt -> psum scratch -> bf16 sbuf casts (DVE); k trimmed to 343
  cols (pad cols of ksb zeroed once per buffer slot)
  v = xt^T@wv -> psum -> bf16 sbuf (DVE)
  simT per (c, head-pair): 2 matmuls into one 2-bank psum tile (j part, i free)
  exp on ACT: one instr per pair (strided 2-bank read) -> expsim_c halves
  attn_c = expsim_c * exp(bias) on DVE (one 1372-col TT per c, bf16 2x)
  AV + rowsum matmuls (M=32, 4-way tile-concurrent) -> aop/rsp psum
  1/rowsum via DVE reciprocal_approx_fast, normalize+cast via one DVE TT
  final = anrm^T @ wout -> psum -> fsb bounce (big half ACT, small half DVE)
"""

import os
import sys
from contextlib import ExitStack

import numpy as np

sys.path.insert(0, "/opt/trn_rl_repo")

import ml_dtypes  # noqa: E402

import concourse.bass as bass  # noqa: E402
import concourse.tile as tile  # noqa: E402
from concourse.tile import add_dep_helper  # noqa: E402
from concourse import bacc, mybir  # noqa: E402
from concourse import bass_utils  # noqa: E402

BF16 = mybir.dt.bfloat16
F32 = mybir.dt.float32

NW = 64          # windows per core
N = 343          # tokens per window
D = 128
H = 4
DH = 32
NP = 384         # padded tokens (zeros beyond 343)
JOFF = [0, 128, 256]

TRACE = False
TRACE_KWARGS = {}

_cache = {}


def _build_kernel():
    nc = bacc.Bacc(
        "TRN2",
        target_bir_lowering=False,
        debug=False,
        enable_asserts=False,
        num_devices=8,
    )
    xt_d = nc.dram_tensor("xt", (NW, D, NP), BF16, kind="ExternalInput").ap()
    wqkv_d = nc.dram_tensor("wqkv", (D, 3 * D), BF16, kind="ExternalInput").ap()
    wout_d = nc.dram_tensor("wout", (D, D), BF16, kind="ExternalInput").ap()
    eb_d = nc.dram_tensor("eb", (D, 3 * H * N), BF16, kind="ExternalInput").ap()
    out_d = nc.dram_tensor("out", (NW, N, D), F32, kind="ExternalOutput").ap()

    with tile.TileContext(nc) as tc:
        with ExitStack() as ctx:
            _body(ctx, tc, out_d, xt_d, wqkv_d, wout_d, eb_d)

    nc.compile()
    return nc


def _chain(insts):
    for a, b in zip(insts[1:], insts[:-1]):
        add_dep_helper(a.ins, b.ins, sync=False, reason="psum accumulation order")


def _body(ctx, tc, out_d, xt_d, wqkv_d, wout_d, eb_d):
    nc = tc.nc

    const = ctx.enter_context(tc.tile_pool(name="const", bufs=1))
    sb = ctx.enter_context(tc.tile_pool(name="sb", bufs=2))
    sb3 = ctx.enter_context(tc.tile_pool(name="sb3", bufs=3))
    ps = ctx.enter_context(tc.tile_pool(name="ps", bufs=1, space="PSUM"))

    # constants
    wqkv = const.tile([D, 3 * D], BF16)
    nc.sync.dma_start(wqkv[:], wqkv_d[:])
    wout = const.tile([D, D], BF16)
    nc.sync.dma_start(wout[:], wout_d[:])
    eb = const.tile([D, 3 * H * N], BF16)
    nc.sync.dma_start(eb[:], eb_d[:])
    ones = const.tile([D, D], BF16)
    nc.vector.memset(ones[:], 1.0)

    for w in range(NW):
        xt = sb.tile([D, NP], BF16, tag="xt")
        nc.sync.dma_start(xt[:], xt_d[w])

        # --- q^T, k^T projections (share one 2-bank tile with the sim rotation) ---
        qkp = ps.tile([D, 2, 512], F32, tag="big", bufs=2)
        nc.tensor.matmul(qkp[:, 0, 0:N], lhsT=wqkv[:, 0:D], rhs=xt[:, 0:N], start=True, stop=True)
        nc.tensor.matmul(qkp[:, 1, 0:N], lhsT=wqkv[:, D:2 * D], rhs=xt[:, 0:N], start=True, stop=True)
        qsb = sb.tile([D, N], BF16, tag="qsb")
        nc.vector.tensor_copy(qsb[:], qkp[:, 0, 0:N])
        ksb = sb.tile([D, NP], BF16, tag="ksb")
        if w < 2:
            # pad cols (343:384) feed sim chunk 2 garbage rows; zero them once
            # per rotating buffer slot (casts below never touch them)
            nc.vector.memset(ksb[:, N:NP], 0.0)
        nc.vector.tensor_copy(ksb[:, 0:N], qkp[:, 1, 0:N])

        # --- v (token-chunk partitions, head dims free) ---
        vpt = ps.tile([D, 2, 512], F32, tag="big", bufs=2)
        v_mms = []
        for c in range(3):
            v_mms.append(nc.tensor.matmul(
                vpt[:, 0, c * D:(c + 1) * D],
                lhsT=xt[:, JOFF[c]:JOFF[c] + D],
                rhs=wqkv[:, 2 * D:3 * D],
                start=(c == 0), stop=(c == 2),
            ))
        _chain(v_mms)
        vsb = sb.tile([D, 3 * D], BF16, tag="vsb")
        cpv = nc.vector.tensor_copy(vsb[:], vpt[:, 0, 0:3 * D])
        add_dep_helper(cpv.ins, v_mms[-1].ins, sync=True, reason="v accum done")

        # --- attention accumulators ---
        aop = ps.tile([D, N], F32, tag="ao", bufs=2, padded_shape=[D, 512])
        rsp = ps.tile([D, N], F32, tag="rs", bufs=2, padded_shape=[D, 512])
        ao_mms = []
        rs_mms = []

        ebr = eb[:].rearrange("p (c h n) -> p c h n", c=3, h=H)
        for c in range(3):
            for r in range(2):
                sim = ps.tile([D, 2, 512], F32, tag="big", bufs=2)
                for hh in range(2):
                    h = 2 * r + hh
                    nc.tensor.matmul(
                        sim[:, hh, 0:N],
                        lhsT=ksb[DH * h:DH * (h + 1), JOFF[c]:JOFF[c] + D],
                        rhs=qsb[DH * h:DH * (h + 1), 0:N],
                        tile_position=(DH * h, 0),
                        start=True, stop=True,
                    )
                expsim = sb3.tile([D, 2, N], BF16, tag="es")
                nc.scalar.activation(
                    expsim[:],
                    sim[:, :, 0:N],
                    mybir.ActivationFunctionType.Exp,
                )
                attn = sb3.tile([D, 2, N], BF16, tag="attn")
                nc.vector.tensor_mul(
                    attn[:], expsim[:], ebr[:, c, 2 * r:2 * r + 2, :],
                )
                for hh in range(2):
                    h = 2 * r + hh
                    ao_mms.append(nc.tensor.matmul(
                        aop[DH * h:DH * (h + 1), :],
                        lhsT=vsb[:, D * c + DH * h:D * c + DH * (h + 1)],
                        rhs=attn[:, hh, :],
                        tile_position=(0, DH * h),
                        start=(c == 0), stop=(c == 2),
                        skip_group_check=True,
                    ))
                    rs_mms.append(nc.tensor.matmul(
                        rsp[DH * h:DH * (h + 1), :],
                        lhsT=ones[:, DH * h:DH * (h + 1)],
                        rhs=attn[:, hh, :],
                        tile_position=(0, DH * h),
                        start=(c == 0), stop=(c == 2),
                        skip_group_check=True,
                    ))
        _chain(ao_mms)
        _chain(rs_mms)

        # --- softmax normalize ---
        recip = sb.tile([D, N], F32, tag="recip")
        rc = nc.vector.reciprocal_approx_fast(recip[:], rsp[:])
        add_dep_helper(rc.ins, rs_mms[-1].ins, sync=True,
                       reason="read rowsums after accumulation closes")
        anrm = sb.tile([D, N], BF16, tag="anrm")
        tt = nc.vector.tensor_mul(anrm[:], aop[:], recip[:])
        add_dep_helper(tt.ins, ao_mms[-1].ins, sync=True,
                       reason="read ao after accumulation closes")

        # --- output projection ---
        fpt = ps.tile([D, 2, 512], F32, tag="big", bufs=2)
        f_mms = []
        for c in range(3):
            jc = min(D, N - JOFF[c])
            f_mms.append(nc.tensor.matmul(
                fpt[0:jc, 0, c * D:(c + 1) * D],
                lhsT=anrm[:, JOFF[c]:JOFF[c] + jc],
                rhs=wout[:],
                start=(c == 0), stop=(c == 2),
                skip_group_check=True,
            ))
        _chain(f_mms)
        fsb = sb.tile([D, 3 * D], F32, tag="fsb")
        cp1 = nc.scalar.copy(fsb[:, 0:2 * D], fpt[:, 0, 0:2 * D])
        add_dep_helper(cp1.ins, f_mms[-1].ins, sync=True,
                       reason="read after accumulation group closes")
        cp2 = nc.vector.tensor_copy(fsb[0:87, 2 * D:3 * D], fpt[0:87, 0, 2 * D:3 * D])
        add_dep_helper(cp2.ins, f_mms[-1].ins, sync=True,
                       reason="read after accumulation group closes")

        dst01 = out_d[w, 0:256, :].rearrange("(c p) d -> p c d", p=D)
        src01 = fsb[:, 0:256].rearrange("p (c d) -> p c d", c=2)
        nc.sync.dma_start(dst01, src01)
        nc.sync.dma_start(out_d[w, 256:343, :], fsb[0:87, 2 * D:3 * D])


def _prep_inputs(x, w_qkv, w_out, bias_table, rel_idx):
    x = np.asarray(x, dtype=np.float32)
    w_qkv = np.asarray(w_qkv, dtype=np.float32)
    w_out = np.asarray(w_out, dtype=np.float32)
    bias_table = np.asarray(bias_table, dtype=np.float32)
    rel_idx = np.asarray(rel_idx)

    scale = DH ** -0.5
    wq = w_qkv[:, 0:D] * scale
    wqkv_s = np.concatenate([wq, w_qkv[:, D:3 * D]], axis=1)
    wqkv_bf = wqkv_s.astype(ml_dtypes.bfloat16)
    wout_bf = w_out.astype(ml_dtypes.bfloat16)

    xr = x.reshape(8 * 64, N, D)
    xtf = np.zeros((8 * 64, D, NP), dtype=np.float32)
    xtf[:, :, 0:N] = xr.transpose(0, 2, 1)
    xt = xtf.astype(ml_dtypes.bfloat16).reshape(8, NW, D, NP)

    bias = bias_table[rel_idx]                     # (i, j, h)
    ebT = np.exp(bias).transpose(1, 2, 0)          # (j, h, i)
    tmp = np.zeros((3 * D, H, N), dtype=np.float32)
    tmp[0:N] = ebT
    eb_arr = np.ascontiguousarray(
        tmp.reshape(3, D, H * N).transpose(1, 0, 2).reshape(D, 3 * H * N)
    ).astype(ml_dtypes.bfloat16)

    in_maps = []
    for core in range(8):
        in_maps.append({
            "xt": np.ascontiguousarray(xt[core]),
            "wqkv": wqkv_bf,
            "wout": wout_bf,
            "eb": eb_arr,
        })
    return in_maps


def kernel(x, w_qkv, w_out, bias_table, rel_idx):
    if "nc" not in _cache:
        _cache["nc"] = _build_kernel()
    nc = _cache["nc"]
    in_maps = _prep_inputs(x, w_qkv, w_out, bias_table, rel_idx)
    res = bass_utils.run_bass_kernel_spmd(
        nc, in_maps, core_ids=list(range(8)), trace=TRACE, **TRACE_KWARGS
    )
    _cache["last_result"] = res
    outs = [res.results[c]["out"] for c in range(8)]
    full = np.concatenate(outs, axis=0)             # (512, 343, 128)
    return full.reshape(1, 8, 8, 8, 7, 7, 7, D).astype(np.float32)


# revision 11
# speedup vs baseline: 1.7388x; 1.7388x over previous
"""Trainium2 Bass kernel for windowed 3D attention (nn_Attention_12927851561046).

512 windows of 343-token, 4-head, 32-dim-per-head attention over d=128.
Pure data parallel: 64 windows per core across 8 NeuronCores.

Layout strategy (per window):
  XT (d=128 partitions, 343 tokens free) bf16
  qT/kT = w^T@XT  -> psum -> cast to bf16 sbuf (128=4h*32dh, 343)
  v    = XT^T@wv  -> psum (t-chunks, 128) -> cast bf16 sbuf (128, 3*128)
  simT chunks (j on partitions, i free), 2 heads per psum tile (128, 686)
  exp on ACT (psum->sbuf bf16), *expbias on DVE/GPSIMD (bf16 2x)
  attnout^T + replicated rowsums via ones-matmul (col-tiled, head-packed)
  1/rowsum via DVE reciprocal_approx_fast, normalize+cast on DVE
  final = anrm^T @ w_out -> psum -> copy -> DMA out
"""

import os
import sys
from contextlib import ExitStack

import numpy as np

sys.path.insert(0, "/opt/trn_rl_repo")

import ml_dtypes  # noqa: E402

import concourse.bass as bass  # noqa: E402
import concourse.tile as tile  # noqa: E402
from concourse.tile import add_dep_helper  # noqa: E402
from concourse import bacc, mybir  # noqa: E402
from concourse import bass_utils  # noqa: E402

BF16 = mybir.dt.bfloat16
F32 = mybir.dt.float32

NW = 64          # windows per core
N = 343          # tokens per window
D = 128
H = 4
DH = 32
NP = 384         # padded tokens (zeros beyond 343)
JOFF = [0, 128, 256]

# bisection toggles
NO_GPSIMD = not bool(int(os.environ.get("K_GPSIMD", "0")))
NO_RECIP_FAST = bool(int(os.environ.get("K_NO_RECIP_FAST", "0")))
NO_TILEPOS_SIM = bool(int(os.environ.get("K_NO_TILEPOS_SIM", "0")))
NO_TILEPOS_AO = bool(int(os.environ.get("K_NO_TILEPOS_AO", "0")))
STAGE = int(os.environ.get("K_STAGE", "0"))  # 0=full, 1..4 truncation
H2ONLY = bool(int(os.environ.get("K_H2ONLY", "0")))  # heads use bases {0,32} only (wrong data, mechanism test)
EXP_SPLIT = bool(int(os.environ.get("K_EXP_SPLIT", "1")))  # per-bank exp reads

# module-level knobs (test.py pokes these)
TRACE = False
TRACE_KWARGS = {}

_cache = {}


def _build_kernel():
    nc = bacc.Bacc(
        "TRN2",
        target_bir_lowering=False,
        debug=False,
        enable_asserts=False,
        num_devices=8,
    )
    xt_d = nc.dram_tensor("xt", (NW, D, NP), BF16, kind="ExternalInput").ap()
    wqkv_d = nc.dram_tensor("wqkv", (D, 3 * D), BF16, kind="ExternalInput").ap()
    wout_d = nc.dram_tensor("wout", (D, D), BF16, kind="ExternalInput").ap()
    eb_d = nc.dram_tensor("eb", (D, 3 * H * N), BF16, kind="ExternalInput").ap()
    out_d = nc.dram_tensor("out", (NW, N, D), F32, kind="ExternalOutput").ap()

    with tile.TileContext(nc) as tc:
        with ExitStack() as ctx:
            _body(ctx, tc, out_d, xt_d, wqkv_d, wout_d, eb_d)

    nc.compile()
    return nc


def _chain(insts):
    for a, b in zip(insts[1:], insts[:-1]):
        add_dep_helper(a.ins, b.ins, sync=False, reason="psum accumulation order")


def _body(ctx, tc, out_d, xt_d, wqkv_d, wout_d, eb_d):
    nc = tc.nc

    const = ctx.enter_context(tc.tile_pool(name="const", bufs=1))
    sb = ctx.enter_context(tc.tile_pool(name="sb", bufs=2))
    ps = ctx.enter_context(tc.tile_pool(name="ps", bufs=1, space="PSUM"))

    # constants
    wqkv = const.tile([D, 3 * D], BF16)
    nc.sync.dma_start(wqkv[:], wqkv_d[:])
    wout = const.tile([D, D], BF16)
    nc.sync.dma_start(wout[:], wout_d[:])
    eb = const.tile([D, 3 * H * N], BF16)
    nc.sync.dma_start(eb[:], eb_d[:])
    ones = const.tile([D, D], BF16)
    nc.vector.memset(ones[:], 1.0)

    for w in range(NW):
        xt = sb.tile([D, NP], BF16, tag="xt")
        nc.sync.dma_start(xt[:], xt_d[w])

        # --- q^T, k^T projections (psum bufs=1 bank, serial reuse) ---
        qp = ps.tile([D, N], F32, tag="qk", bufs=1, padded_shape=[D, 512])
        nc.tensor.matmul(qp[:], lhsT=wqkv[:, 0:D], rhs=xt[:, 0:N], start=True, stop=True)
        qsb = sb.tile([D, N], BF16, tag="qsb")
        nc.vector.tensor_copy(qsb[:], qp[:])          # DVE cast

        kp = ps.tile([D, N], F32, tag="qk", bufs=1, padded_shape=[D, 512])
        nc.tensor.matmul(kp[:], lhsT=wqkv[:, D:2 * D], rhs=xt[:, 0:N], start=True, stop=True)
        ksb = sb.tile([D, NP], BF16, tag="ksb")
        if w < 2:
            # pad cols feed sim chunk-2 garbage rows; zero once per buffer slot
            nc.vector.memset(ksb[:, N:NP], 0.0)
        nc.scalar.copy(ksb[:, 0:N], kp[:])            # ACT cast

        # --- v (token-chunk partitions, head dims free) ---
        vp = ps.tile([D, 3 * D], F32, tag="v", bufs=1, padded_shape=[D, 512])
        v_mms = []
        for c in range(3):
            v_mms.append(nc.tensor.matmul(
                vp[:, c * D:(c + 1) * D],
                lhsT=xt[:, JOFF[c]:JOFF[c] + D],
                rhs=wqkv[:, 2 * D:3 * D],
                start=(c == 0), stop=(c == 2),
            ))
        _chain(v_mms)
        vsb = sb.tile([D, 3 * D], BF16, tag="vsb")
        nc.vector.tensor_copy(vsb[:], vp[:])          # DVE cast

        if STAGE == 1:
            fsb = sb.tile([D, 3 * D], F32, tag="fsb")
            nc.vector.tensor_copy(fsb[:], vp[:])
            dst01 = out_d[w, 0:256, :].rearrange("(c p) d -> p c d", p=D)
            src01 = fsb[:, 0:256].rearrange("p (c d) -> p c d", c=2)
            nc.sync.dma_start(dst01, src01)
            nc.sync.dma_start(out_d[w, 256:343, :], fsb[0:87, 2 * D:3 * D])
            continue

        # --- attention accumulators ---
        aop = ps.tile([D, N], F32, tag="ao", bufs=1, padded_shape=[D, 512])
        rsp = ps.tile([D, N], F32, tag="rs", bufs=1, padded_shape=[D, 512])
        ao_mms = []
        rs_mms = []

        for c in range(3):
            expsim = sb.tile([D, H * N], BF16, tag="es")
            for r in range(2):
                s = ps.tile([D, 2, 512], F32, tag="sim", bufs=2)
                for hh in range(2):
                    h = 2 * r + hh
                    nc.tensor.matmul(
                        s[:, hh, 0:N],
                        lhsT=ksb[DH * h:DH * (h + 1), JOFF[c]:JOFF[c] + D],
                        rhs=qsb[DH * h:DH * (h + 1), 0:N],
                        tile_position=(DH * h, 0),
                        start=True, stop=True,
                    )
                nc.scalar.activation(
                    expsim[:, N * 2 * r:N * (2 * r + 2)],
                    s[:, :, 0:N],
                    mybir.ActivationFunctionType.Exp,
                )
            if STAGE == 2:
                continue
            attn = sb.tile([D, H * N], BF16, tag="attn")
            eng = nc.gpsimd if (c == 2 and not NO_GPSIMD) else nc.vector
            eng.tensor_mul(attn[:], expsim[:], eb[:, H * N * c:H * N * (c + 1)])

            if STAGE == 2:
                continue
            for h in range(H):
                ao_mms.append(nc.tensor.matmul(
                    aop[DH * h:DH * (h + 1), :],
                    lhsT=vsb[:, D * c + DH * h:D * c + DH * (h + 1)],
                    rhs=attn[:, N * h:N * (h + 1)],
                    tile_position=None if NO_TILEPOS_AO else (0, DH * h),
                    start=(c == 0), stop=(c == 2),
                    skip_group_check=True,
                ))
            for h in range(H):
                rs_mms.append(nc.tensor.matmul(
                    rsp[DH * h:DH * (h + 1), :],
                    lhsT=ones[:, DH * h:DH * (h + 1)],
                    rhs=attn[:, N * h:N * (h + 1)],
                    tile_position=None if NO_TILEPOS_AO else (0, DH * h),
                    start=(c == 0), stop=(c == 2),
                    skip_group_check=True,
                ))

        if STAGE == 2:
            fsb = sb.tile([D, 3 * D], F32, tag="fsb")
            nc.vector.tensor_copy(fsb[:], expsim[:, 0:3 * D])
            dst01 = out_d[w, 0:256, :].rearrange("(c p) d -> p c d", p=D)
            src01 = fsb[:, 0:256].rearrange("p (c d) -> p c d", c=2)
            nc.sync.dma_start(dst01, src01)
            nc.sync.dma_start(out_d[w, 256:343, :], fsb[0:87, 2 * D:3 * D])
            continue
        _chain(ao_mms)
        _chain(rs_mms)

        if STAGE == 3:
            fsb = sb.tile([D, 3 * D], F32, tag="fsb")
            nc.vector.tensor_copy(fsb[:], aop[:, 0:3 * D])
            nc.vector.tensor_copy(fsb[0:1, 0:1], rsp[0:1, 0:1])
            dst01 = out_d[w, 0:256, :].rearrange("(c p) d -> p c d", p=D)
            src01 = fsb[:, 0:256].rearrange("p (c d) -> p c d", c=2)
            nc.sync.dma_start(dst01, src01)
            nc.sync.dma_start(out_d[w, 256:343, :], fsb[0:87, 2 * D:3 * D])
            continue
        # --- softmax normalize ---
        recip = sb.tile([D, N], F32, tag="recip")
        if NO_RECIP_FAST:
            nc.vector.reciprocal(recip[:], rsp[:])
        else:
            nc.vector.reciprocal_approx_fast(recip[:], rsp[:])
        anrm = sb.tile([D, N], BF16, tag="anrm")
        nc.vector.tensor_mul(anrm[:], aop[:], recip[:])

        # --- output projection ---
        fp = ps.tile([D, 3 * D], F32, tag="ao", bufs=1, padded_shape=[D, 512])
        f_mms = []
        for c in range(3):
            jc = min(D, N - JOFF[c])
            f_mms.append(nc.tensor.matmul(
                fp[0:jc, c * D:(c + 1) * D],
                lhsT=anrm[:, JOFF[c]:JOFF[c] + jc],
                rhs=wout[:],
                start=(c == 0), stop=(c == 2),
                skip_group_check=True,
            ))
        _chain(f_mms)
        fsb = sb.tile([D, 3 * D], F32, tag="fsb")
        cp1 = nc.vector.tensor_copy(fsb[:, 0:2 * D], fp[:, 0:2 * D])
        add_dep_helper(cp1.ins, f_mms[-1].ins, sync=True,
                       reason="read after accumulation group closes")
        nc.vector.tensor_copy(fsb[0:87, 2 * D:3 * D], fp[0:87, 2 * D:3 * D])

        dst01 = out_d[w, 0:256, :].rearrange("(c p) d -> p c d", p=D)
        src01 = fsb[:, 0:256].rearrange("p (c d) -> p c d", c=2)
        nc.sync.dma_start(dst01, src01)
        nc.sync.dma_start(out_d[w, 256:343, :], fsb[0:87, 2 * D:3 * D])


def _prep_inputs(x, w_qkv, w_out, bias_table, rel_idx):
    x = np.asarray(x, dtype=np.float32)
    w_qkv = np.asarray(w_qkv, dtype=np.float32)
    w_out = np.asarray(w_out, dtype=np.float32)
    bias_table = np.asarray(bias_table, dtype=np.float32)
    rel_idx = np.asarray(rel_idx)

    scale = DH ** -0.5
    wq = w_qkv[:, 0:D] * scale
    wqkv_s = np.concatenate([wq, w_qkv[:, D:3 * D]], axis=1)
    wqkv_bf = wqkv_s.astype(ml_dtypes.bfloat16)
    wout_bf = w_out.astype(ml_dtypes.bfloat16)

    xr = x.reshape(8 * 64, N, D)
    xtf = np.zeros((8 * 64, D, NP), dtype=np.float32)
    xtf[:, :, 0:N] = xr.transpose(0, 2, 1)
    xt = xtf.astype(ml_dtypes.bfloat16).reshape(8, NW, D, NP)

    bias = bias_table[rel_idx]                     # (i, j, h)
    ebT = np.exp(bias).transpose(1, 2, 0)          # (j, h, i)
    tmp = np.zeros((3 * D, H, N), dtype=np.float32)
    tmp[0:N] = ebT
    eb_arr = np.ascontiguousarray(
        tmp.reshape(3, D, H * N).transpose(1, 0, 2).reshape(D, 3 * H * N)
    ).astype(ml_dtypes.bfloat16)

    in_maps = []
    for core in range(8):
        in_maps.append({
            "xt": np.ascontiguousarray(xt[core]),
            "wqkv": wqkv_bf,
            "wout": wout_bf,
            "eb": eb_arr,
        })
    return in_maps


def kernel(x, w_qkv, w_out, bias_table, rel_idx):
    if "nc" not in _cache:
        _cache["nc"] = _build_kernel()
    nc = _cache["nc"]
    in_maps = _prep_inputs(x, w_qkv, w_out, bias_table, rel_idx)
    res = bass_utils.run_bass_kernel_spmd(
        nc, in_maps, core_ids=list(range(8)), trace=TRACE, **TRACE_KWARGS
    )
    _cache["last_result"] = res
    outs = [res.results[c]["out"] for c in range(8)]
    full = np.concatenate(outs, axis=0)             # (512, 343, 128)
    return full.reshape(1, 8, 8, 8, 7, 7, 7, D).astype(np.float32)

